# revision 26
# baseline (speedup 1.0000x reference)
"""Sliding-window GQA attention block (RoPE + QKV proj + SDPA + O proj) on 8
Trainium2 NeuronCores, head-sharded (1 kv-head group = 8 q-heads per core).

Contract: kernel(**inputs) takes the FULL unsharded inputs from
setup_inputs() and returns the FULL [1, 2048, 2880] output.

v2 design (single-phase pipeline, bf16 datapath, S-layout softmax):
  - All device inputs are bf16 (xT with a ones-row folding biases into the
    matmul, per-core transposed weight slices packed as [DP, 640] = q|k|v,
    RoPE cos/sin with head-dim permutation + sign folded, 0/1 masks).
  - QKV projections stream per 512-seq chunk, mt-major (one PSUM bank per
    output m-tile, rotating through a shared 4-bank pool); RoPE epilogue on
    DVE in bf16; qT/kT kept in [head_dim, seq] bf16; v transposed to natural
    [seq, head_dim] via PE.
  - Attention per 128-query tile j in S layout: scores [q, 256 keys]
    (keys = tiles j-1, j) with the additive -1e30 window mask folded in as
    a second matmul (id128^T @ mask) into the same PSUM accumulation; the
    scalar engine then does exp straight from PSUM with accum_out giving
    the softmax denominator [128,1] per head; one reciprocal [128,8] per
    tile; normalize e, transpose it on the PE, AV accumulates the
    normalized attention output [hd, q] directly.
  - O-projection of tile j-1 is emitted between tile j's scores and its
    normalize/AV stage: it fills the PE while the exp/mask chain drains,
    keeping the tensor engine continuously busy (full-clock p-state).
  - PSUM: shared tag (qkv accum + v-transposes + score pairs + eT) 4 banks,
    attention-out po 2 banks, O-proj 2 banks = 8 banks exactly.
  - Per-core partial [2048, 2880] fp32 returned; host sums 8 partials + bias.
"""
import sys

sys.path.insert(0, "/opt/trn_rl_repo")

import numpy as np

import concourse.bass as bass  # noqa: F401  (import keeps bass registered)
import concourse.tile as tile
from concourse import bacc, mybir
from concourse.bass_utils import run_bass_kernel_spmd

B, S, D = 1, 2048, 2880
H, KVH, HD = 64, 8, 64
WINDOW = 128
N_CORES = 8
DP = 2944  # padded contraction dim: 23 * 128 (2880 data + 1 ones row + pad)
KT = DP // 128  # 23 contraction tiles
NQT = S // 128  # 16 seq tiles
OCH = 480  # O-proj free chunk (6 * 480 = 2880)

F32 = mybir.dt.float32
BF16 = mybir.dt.bfloat16

# head-dim permutation: pairs (t, t+32) adjacent -> rotate-half partner is
# the neighbouring partition (stream_shuffle mask i^1 within quadrants)
PERM = np.empty(HD, dtype=np.int64)
PERM[0::2] = np.arange(32)
PERM[1::2] = np.arange(32) + 32

_COMPILED = None


def _build():
    nc = bacc.Bacc("TRN2", target_bir_lowering=False, debug=False)

    xT_d = nc.dram_tensor("xT", [DP, S], BF16, kind="ExternalInput").ap()
    w_d = nc.dram_tensor("wqkv", [DP, 640], BF16, kind="ExternalInput").ap()
    wo_d = nc.dram_tensor("wo", [512, D], BF16, kind="ExternalInput").ap()
    cos_d = nc.dram_tensor("cosT", [128, S], BF16, kind="ExternalInput").ap()
    sin_d = nc.dram_tensor("sinTs", [128, S], BF16, kind="ExternalInput").ap()
    mm_d = nc.dram_tensor("m_main", [128, 256], BF16, kind="ExternalInput").ap()
    md_d = nc.dram_tensor("m_diag", [128, 128], BF16, kind="ExternalInput").ap()
    id64_d = nc.dram_tensor("id64", [64, 64], BF16, kind="ExternalInput").ap()
    id128_d = nc.dram_tensor("id128", [128, 128], BF16, kind="ExternalInput").ap()
    out_d = nc.dram_tensor("partial", [S, D], F32, kind="ExternalOutput").ap()

    Exp = mybir.ActivationFunctionType.Exp
    MULT = mybir.AluOpType.mult
    ADD = mybir.AluOpType.add
    SHUF_MASK = [i ^ 1 for i in range(32)]

    with tile.TileContext(nc) as tc:
        with (
            tc.tile_pool(name="constp", bufs=1) as constp,
            tc.tile_pool(name="wpool", bufs=1) as wpool,
            tc.tile_pool(name="xsp", bufs=46) as xsp,
            tc.tile_pool(name="qkvp", bufs=1) as qkvp,
            tc.tile_pool(name="workp", bufs=3) as workp,
            tc.tile_pool(name="epool", bufs=3) as epool,
            tc.tile_pool(name="empool", bufs=10) as empool,
            tc.tile_pool(name="drp", bufs=3) as drp,
            tc.tile_pool(name="outsp", bufs=8) as outsp,
            tc.tile_pool(name="psQS", bufs=3, space="PSUM") as psQS,
            tc.tile_pool(name="psPO", bufs=1, space="PSUM") as psPO,
            tc.tile_pool(name="psPP", bufs=2, space="PSUM") as psPP,
        ):
            # ---- constants ----
            cos_t = constp.tile([128, S], BF16)
            sin_t = constp.tile([128, S], BF16)
            mm_t = constp.tile([128, 256], BF16)
            md_t = constp.tile([128, 128], BF16)
            id64_t = constp.tile([64, 64], BF16)
            id128_t = constp.tile([128, 128], BF16)
            nc.sync.dma_start(cos_t[:], cos_d[:])
            nc.sync.dma_start(sin_t[:], sin_d[:])
            nc.sync.dma_start(mm_t[:], mm_d[:])
            nc.sync.dma_start(md_t[:], md_d[:])
            nc.sync.dma_start(id64_t[:], id64_d[:])
            nc.sync.dma_start(id128_t[:], id128_d[:])

            # ---- weight + first x chunk DMAs, issue-interleaved so the
            # first contraction tiles land first ----
            w_sb = []
            x_sb = [[None] * KT for _ in range(4)]
            for k in range(KT):
                w_t = wpool.tile([128, 640], BF16, name=f"w{k}")
                nc.sync.dma_start(w_t[:], w_d[128 * k : 128 * (k + 1), :])
                w_sb.append(w_t)
                x_t = xsp.tile([128, 512], BF16, name="x_t", tag="x")
                nc.sync.dma_start(x_t[:], xT_d[128 * k : 128 * (k + 1), 0:512])
                x_sb[0][k] = x_t
            wo_sb = []
            for t in range(4):
                w_t = wpool.tile([128, D], BF16, name=f"wo{t}")
                nc.sync.dma_start(w_t[:], wo_d[128 * t : 128 * (t + 1), :])
                wo_sb.append(w_t)

            def prefetch_x(sq):
                for k in range(KT):
                    x_t = xsp.tile([128, 512], BF16, name="x_t", tag="x")
                    nc.sync.dma_start(
                        x_t[:], xT_d[128 * k : 128 * (k + 1), 512 * sq : 512 * (sq + 1)]
                    )
                    x_sb[sq][k] = x_t

            # ---- persistent bf16 activation tensors ----
            qTm = [qkvp.tile([128, S], BF16, name=f"qTm{t}") for t in range(4)]
            kT2 = qkvp.tile([128, S], BF16, name="kT2")
            vT = qkvp.tile([64, S], BF16, name="vT")
            # v_lo = [v | 0], v_hi = [0 | v]: AV for an even head lands in
            # PSUM rows 0:64, its odd partner in rows 64:128 of the same
            # region -> head pairs stack in place, one copy per pair
            v_lo = [qkvp.tile([128, 128], BF16, name=f"vl{i}") for i in range(NQT)]
            v_hi = [qkvp.tile([128, 128], BF16, name=f"vh{i}") for i in range(NQT)]
            attn_oT = [qkvp.tile([128, S], BF16, name=f"aoT{t}") for t in range(4)]
            for i in range(NQT):
                nc.vector.memset(v_lo[i][:, 64:128], 0.0)
                nc.vector.memset(v_hi[i][:, 0:64], 0.0)

            def rope_q(ps, mt, c0):
                # DVE casts the PSUM once; the SBUF-only bf16 muls/adds go
                # to the idle GpSimd engine
                t_all = workp.tile([128, 512], BF16, tag="ra", name="t_all")
                nc.vector.tensor_copy(t_all[:], ps[:])
                t_shuf = workp.tile([128, 512], BF16, tag="rb", name="t_shuf")
                nc.vector.stream_shuffle(t_shuf[:], t_all[:], SHUF_MASK)
                t_cos = workp.tile([128, 512], BF16, tag="rc", name="t_cos")
                nc.gpsimd.tensor_mul(t_cos[:], t_all[:], cos_t[:, c0 : c0 + 512])
                t_sin = workp.tile([128, 512], BF16, tag="rd", name="t_sin")
                nc.gpsimd.tensor_mul(t_sin[:], t_shuf[:], sin_t[:, c0 : c0 + 512])
                nc.gpsimd.tensor_add(qTm[mt][:, c0 : c0 + 512], t_cos[:], t_sin[:])

            def rope_kv(ps, c0):
                t_all = workp.tile([128, 512], BF16, tag="ra", name="t_allk")
                nc.vector.tensor_copy(t_all[0:64, :], ps[0:64, :])
                t_shuf = workp.tile([128, 512], BF16, tag="rb", name="t_shufk")
                nc.vector.stream_shuffle(t_shuf[0:64, :], t_all[0:64, :], SHUF_MASK)
                t_cos = workp.tile([128, 512], BF16, tag="rc", name="t_cosk")
                nc.gpsimd.tensor_mul(
                    t_cos[0:64, :], t_all[0:64, :], cos_t[0:64, c0 : c0 + 512]
                )
                t_sin = workp.tile([128, 512], BF16, tag="rd", name="t_sink")
                nc.gpsimd.tensor_mul(
                    t_sin[0:64, :], t_shuf[0:64, :], sin_t[0:64, c0 : c0 + 512]
                )
                nc.gpsimd.tensor_add(
                    kT2[0:64, c0 : c0 + 512], t_cos[0:64, :], t_sin[0:64, :]
                )
                # partition-shifted write stays on the vector engine
                nc.vector.tensor_add(
                    kT2[64:128, c0 : c0 + 512], t_cos[0:64, :], t_sin[0:64, :]
                )
                nc.vector.tensor_copy(vT[:, c0 : c0 + 512], ps[64:128, :])

            def emit_oproj(j, ch0, ch1):
                for ch in range(ch0, ch1):
                    pp = psPP.tile([128, OCH], F32, name="pp", tag="pp")
                    for t in range(4):
                        nc.tensor.matmul(
                            pp[:],
                            attn_oT[t][:, 128 * j : 128 * (j + 1)],
                            wo_sb[t][:, OCH * ch : OCH * (ch + 1)],
                            start=(t == 0),
                            stop=(t == 3),
                        )
                    osb = outsp.tile([128, OCH], F32, tag="osb", name="osb")
                    if ch in (0, 3):
                        nc.scalar.copy(osb[:], pp[:])
                    else:
                        nc.vector.tensor_copy(osb[:], pp[:])
                    nc.sync.dma_start(
                        out_d[128 * j : 128 * (j + 1), OCH * ch : OCH * (ch + 1)],
                        osb[:],
                    )

            for sq in range(4):
                c0 = 512 * sq
                if sq < 3:
                    prefetch_x(sq + 1)
                # ---- QKV projections, mt-major ----
                for mt in range(4):
                    ps = psQS.tile([128, 512], F32, tag="sp", name="ps_q")
                    for k in range(KT):
                        nc.tensor.matmul(
                            ps[:],
                            w_sb[k][:, 128 * mt : 128 * (mt + 1)],
                            x_sb[sq][k][:],
                            start=(k == 0),
                            stop=(k == KT - 1),
                        )
                    rope_q(ps, mt, c0)
                ps = psQS.tile([128, 512], F32, tag="sp", name="ps_kv")
                for k in range(KT):
                    nc.tensor.matmul(
                        ps[:],
                        w_sb[k][:, 512:640],
                        x_sb[sq][k][:],
                        start=(k == 0),
                        stop=(k == KT - 1),
                    )
                rope_kv(ps, c0)
                # ---- v transposes to natural [seq, hd] -> v_lo / v_hi ----
                for i in range(4 * sq, 4 * sq + 4):
                    tr = psQS.tile([128, 64], BF16, tag="sp", name="vtr")
                    nc.tensor.transpose(
                        tr[:], vT[:, 128 * i : 128 * (i + 1)], id64_t[:]
                    )
                    nc.scalar.copy(v_lo[i][:, 0:64], tr[:])
                    nc.scalar.copy(v_hi[i][:, 64:128], tr[:])

                # ---- attention for the 4 query tiles of this chunk ----
                for j in range(4 * sq, 4 * sq + 4):
                    W = 128 if j == 0 else 256
                    kc0 = 0 if j == 0 else 128 * (j - 1)
                    mask = md_t if j == 0 else mm_t
                    den = drp.tile([128, 8], F32, tag="den", name="den")
                    e_ms = []
                    for h in range(8):
                        t, rb = h // 2, 64 * (h % 2)
                        if h % 2 == 0:
                            s_pair = psQS.tile(
                                [128, 512], F32, tag="sp", name="s_pair"
                            )
                        sl = s_pair[:, 256 * (h % 2) : 256 * (h % 2) + W]
                        nc.tensor.matmul(
                            sl,
                            qTm[t][rb : rb + 64, 128 * j : 128 * (j + 1)],
                            kT2[rb : rb + 64, kc0 : kc0 + W],
                            start=True,
                            stop=False,
                        )
                        # additive -1e30 window mask via id128^T @ mask
                        nc.tensor.matmul(
                            sl,
                            id128_t[:],
                            mask[:, 0:W],
                            start=False,
                            stop=True,
                        )
                        # masked exp straight from PSUM; accum_out = softmax den
                        e_m = empool.tile([128, 256], BF16, tag="em", name="e_m")
                        nc.scalar.activation(
                            e_m[:, 0:W],
                            sl,
                            Exp,
                            scale=0.125,
                            accum_out=den[:, h : h + 1],
                        )
                        e_ms.append(e_m)
                        # PE filler between score pairs: previous O-projection
                        if j > 0:
                            if h == 3:
                                emit_oproj(j - 1, 0, 3)
                            elif h == 7:
                                emit_oproj(j - 1, 3, 6)
                    rec = drp.tile([128, 8], F32, tag="rec", name="rec")
                    nc.vector.reciprocal(rec[:], den[:])
                    # one [128,512] PSUM region holds all 8 heads: pair
                    # (2t, 2t+1) stacks into rows 0:64 / 64:128 of quarter t
                    po = psPO.tile([128, 512], F32, tag="po", name="po")
                    for h in range(8):
                        t = h // 2
                        hp = h % 2  # position within the eT pair
                        e_n = epool.tile([128, 256], BF16, tag="en", name="e_n")
                        nc.vector.tensor_scalar_mul(
                            e_n[:, 0:W], e_ms[h][:, 0:W], rec[:, h : h + 1]
                        )
                        if hp == 0:
                            eT_ps = psQS.tile([128, 512], BF16, tag="et", name="eT_ps", bufs=2)
                        for half in range(W // 128):
                            nc.tensor.transpose(
                                eT_ps[:, 256 * hp + 128 * half : 256 * hp + 128 * (half + 1)],
                                e_n[:, 128 * half : 128 * (half + 1)],
                                id128_t[:],
                            )
                        if hp == 1:
                            eT_sb = epool.tile(
                                [128, 512], BF16, tag="et", name="eT_sb"
                            )
                            if W == 256:
                                nc.vector.tensor_copy(eT_sb[:], eT_ps[:])
                            else:
                                nc.vector.tensor_copy(
                                    eT_sb[:, 0:128], eT_ps[:, 0:128]
                                )
                                nc.vector.tensor_copy(
                                    eT_sb[:, 256:384], eT_ps[:, 256:384]
                                )
                            # AV for the pair: even head -> rows 0:64 (v_lo),
                            # odd head -> rows 64:128 (v_hi), same quarter
                            for h2 in (h - 1, h):
                                hp2 = h2 % 2
                                vv = [v_lo, v_hi][hp2]
                                dst = po[:, 128 * t : 128 * (t + 1)]
                                if j > 0:
                                    nc.tensor.matmul(
                                        dst,
                                        vv[j - 1][:],
                                        eT_sb[:, 256 * hp2 : 256 * hp2 + 128],
                                        start=(hp2 == 0),
                                        stop=False,
                                    )
                                    nc.tensor.matmul(
                                        dst,
                                        vv[j][:],
                                        eT_sb[:, 256 * hp2 + 128 : 256 * hp2 + 256],
                                        start=False,
                                        stop=(hp2 == 1),
                                    )
                                else:
                                    nc.tensor.matmul(
                                        dst,
                                        vv[0][:],
                                        eT_sb[:, 256 * hp2 : 256 * hp2 + 128],
                                        start=(hp2 == 0),
                                        stop=(hp2 == 1),
                                    )
                    for t in range(4):
                        nc.scalar.copy(
                            attn_oT[t][:, 128 * j : 128 * (j + 1)],
                            po[:, 128 * t : 128 * (t + 1)],
                        )
            emit_oproj(NQT - 1, 0, 6)
    nc.compile()
    return nc


def _prep_inputs(x, rope_cache, wq_w, wq_b, wk_w, wk_b, wv_w, wv_b, wo_w):
    """Build the shared + per-core input maps (all bf16 on device)."""
    import ml_dtypes

    bf = ml_dtypes.bfloat16
    xT = np.zeros((DP, S), dtype=np.float32)
    xT[0:D, :] = np.ascontiguousarray(x[0].T)
    xT[D, :] = 1.0  # bias row
    xT = xT.astype(bf)

    cos = np.asarray(rope_cache[:, 0, :], dtype=np.float32)  # [S, 64]
    sin = np.asarray(rope_cache[:, 1, :], dtype=np.float32)
    cosP = cos[:, PERM].T  # [64, S] permuted head-dim rows
    sinP = sin[:, PERM].T
    sign = np.where(PERM < 32, -1.0, 1.0).astype(np.float32)[:, None]
    sinPs = sinP * sign
    cosT = np.concatenate([cosP, cosP], axis=0).astype(bf)  # [128, S]
    sinTs = np.concatenate([sinPs, sinPs], axis=0).astype(bf)

    # S-layout additive masks: query a on partitions, key b on free.
    a_idx = np.arange(128)[:, None]
    b_idx = np.arange(256)[None, :]
    left = (b_idx < 128) & (b_idx > a_idx)
    right = (b_idx >= 128) & ((b_idx - 128) <= a_idx)
    m_main = np.where(left | right, 0.0, -1e30).astype(bf)  # [128, 256], j >= 1
    m_diag = np.where(b_idx[:, :128] <= a_idx, 0.0, -1e30).astype(bf)  # j == 0

    id64 = np.eye(64, dtype=np.float32).astype(bf)
    id128 = np.eye(128, dtype=np.float32).astype(bf)

    shared = dict(
        xT=xT, cosT=cosT, sinTs=sinTs, m_main=m_main, m_diag=m_diag,
        id64=id64, id128=id128,
    )

    in_maps = []
    for c in range(N_CORES):
        # wq slice: q heads [8c, 8c+8), head-dim permuted, transposed, bias row
        wq_rows = []
        bq_rows = []
        for hh in range(8):
            g = 8 * c + hh
            wq_rows.append(wq_w[64 * g + PERM, :])  # [64, D]
            bq_rows.append(wq_b[64 * g + PERM])
        wq_slice = np.concatenate(wq_rows, axis=0)  # [512, D]
        bq_slice = np.concatenate(bq_rows, axis=0)  # [512]

        wk_slice = wk_w[64 * c + PERM, :]  # [64, D] permuted
        bk_slice = wk_b[64 * c + PERM]
        wv_slice = wv_w[64 * c : 64 * (c + 1), :]  # unpermuted
        bv_slice = wv_b[64 * c : 64 * (c + 1)]

        w_t = np.zeros((DP, 640), dtype=np.float32)
        w_t[0:D, 0:512] = wq_slice.T
        w_t[D, 0:512] = bq_slice
        w_t[0:D, 512:576] = wk_slice.T
        w_t[D, 512:576] = bk_slice
        w_t[0:D, 576:640] = wv_slice.T
        w_t[D, 576:640] = bv_slice

        wo_t = np.ascontiguousarray(
            wo_w[:, 512 * c : 512 * (c + 1)].T
        ).astype(bf)  # [512, D]

        in_maps.append(dict(shared, wqkv=w_t.astype(bf), wo=wo_t))
    return in_maps


def kernel(
    x,
    rope_cache,
    wq_w,
    wq_b,
    wk_w,
    wk_b,
    wv_w,
    wv_b,
    wo_w,
    wo_b,
):
    global _COMPILED
    x = np.asarray(x, dtype=np.float32)
    rope_cache = np.asarray(rope_cache, dtype=np.float32)
    wq_w = np.asarray(wq_w, dtype=np.float32)
    wq_b = np.asarray(wq_b, dtype=np.float32)
    wk_w = np.asarray(wk_w, dtype=np.float32)
    wk_b = np.asarray(wk_b, dtype=np.float32)
    wv_w = np.asarray(wv_w, dtype=np.float32)
    wv_b = np.asarray(wv_b, dtype=np.float32)
    wo_w = np.asarray(wo_w, dtype=np.float32)
    wo_b = np.asarray(wo_b, dtype=np.float32)

    if _COMPILED is None:
        _COMPILED = _build()
    nc = _COMPILED

    in_maps = _prep_inputs(x, rope_cache, wq_w, wq_b, wk_w, wk_b, wv_w, wv_b, wo_w)
    res = run_bass_kernel_spmd(nc, in_maps, core_ids=list(range(N_CORES)), trace=False)
    out = np.zeros((S, D), dtype=np.float32)
    for c in range(N_CORES):
        out += res.results[c]["partial"]
    out += wo_b[None, :]
    return out.reshape(B, S, D).astype(np.float32)


# expose the compiled-module runner for test harnesses that want tracing
def run_traced(**inputs):
    global _COMPILED
    if _COMPILED is None:
        _COMPILED = _build()
    in_maps = _prep_inputs(
        np.asarray(inputs["x"], np.float32),
        np.asarray(inputs["rope_cache"], np.float32),
        np.asarray(inputs["wq_w"], np.float32),
        np.asarray(inputs["wq_b"], np.float32),
        np.asarray(inputs["wk_w"], np.float32),
        np.asarray(inputs["wk_b"], np.float32),
        np.asarray(inputs["wv_w"], np.float32),
        np.asarray(inputs["wv_b"], np.float32),
        np.asarray(inputs["wo_w"], np.float32),
    )
    res = run_bass_kernel_spmd(
        _COMPILED, in_maps, core_ids=list(range(N_CORES)), trace=True
    )
    out = np.zeros((S, D), dtype=np.float32)
    for c in range(N_CORES):
        out += res.results[c]["partial"]
    out += np.asarray(inputs["wo_b"], np.float32)[None, :]
    return out.reshape(B, S, D).astype(np.float32), res


# revision 32
# speedup vs baseline: 1.1271x; 1.1271x over previous
"""Sliding-window GQA attention block (RoPE + QKV proj + SDPA + O proj) on 8
Trainium2 NeuronCores, head-sharded (1 kv-head group = 8 q-heads per core).

Contract: kernel(**inputs) takes the FULL unsharded inputs from
setup_inputs() and returns the FULL [1, 2048, 2880] output.

v2 design (single-phase pipeline, bf16 datapath, S-layout softmax):
  - All device inputs are bf16 (xT with a ones-row folding biases into the
    matmul, per-core transposed weight slices packed as [DP, 640] = q|k|v,
    RoPE cos/sin with head-dim permutation + sign folded, 0/1 masks).
  - QKV projections stream per 512-seq chunk, mt-major (one PSUM bank per
    output m-tile, rotating through a shared 4-bank pool); RoPE epilogue on
    DVE in bf16; qT/kT kept in [head_dim, seq] bf16; v transposed to natural
    [seq, head_dim] via PE.
  - Attention per 128-query tile j in S layout: scores [q, 256 keys]
    (keys = tiles j-1, j) with the additive -1e30 window mask folded in as
    a second matmul (id128^T @ mask) into the same PSUM accumulation; the
    scalar engine then does exp straight from PSUM with accum_out giving
    the softmax denominator [128,1] per head; one reciprocal [128,8] per
    tile; normalize e, transpose it on the PE, AV accumulates the
    normalized attention output [hd, q] directly.
  - O-projection of tile j-1 is emitted between tile j's scores and its
    normalize/AV stage: it fills the PE while the exp/mask chain drains,
    keeping the tensor engine continuously busy (full-clock p-state).
  - PSUM: shared tag (qkv accum + v-transposes + score pairs + eT) 4 banks,
    attention-out po 2 banks, O-proj 2 banks = 8 banks exactly.
  - Per-core partial [2048, 2880] fp32 returned; host sums 8 partials + bias.
"""
import sys

sys.path.insert(0, "/opt/trn_rl_repo")

import numpy as np

import concourse.bass as bass  # noqa: F401  (import keeps bass registered)
import concourse.tile as tile
from concourse import bacc, mybir
from concourse.bass_utils import run_bass_kernel_spmd

B, S, D = 1, 2048, 2880
H, KVH, HD = 64, 8, 64
WINDOW = 128
N_CORES = 8
DP = 2944  # padded contraction dim: 23 * 128 (2880 data + 1 ones row + pad)
KT = DP // 128  # 23 contraction tiles
NQT = S // 128  # 16 seq tiles
OCH = 480  # O-proj free chunk (6 * 480 = 2880)

F32 = mybir.dt.float32
BF16 = mybir.dt.bfloat16

# head-dim permutation: pairs (t, t+32) adjacent -> rotate-half partner is
# the neighbouring partition (stream_shuffle mask i^1 within quadrants)
PERM = np.empty(HD, dtype=np.int64)
PERM[0::2] = np.arange(32)
PERM[1::2] = np.arange(32) + 32

_COMPILED = None


def _build():
    nc = bacc.Bacc("TRN2", target_bir_lowering=False, debug=False)

    xT_d = nc.dram_tensor("xT", [DP, S], BF16, kind="ExternalInput").ap()
    w_d = nc.dram_tensor("wqkv", [DP, 640], BF16, kind="ExternalInput").ap()
    wo_d = nc.dram_tensor("wo", [512, D], BF16, kind="ExternalInput").ap()
    cos_d = nc.dram_tensor("cosT", [128, S], BF16, kind="ExternalInput").ap()
    sin_d = nc.dram_tensor("sinTs", [128, S], BF16, kind="ExternalInput").ap()
    mm_d = nc.dram_tensor("m_main", [128, 256], BF16, kind="ExternalInput").ap()
    md_d = nc.dram_tensor("m_diag", [128, 128], BF16, kind="ExternalInput").ap()
    id64_d = nc.dram_tensor("id64", [64, 64], BF16, kind="ExternalInput").ap()
    id128_d = nc.dram_tensor("id128", [128, 128], BF16, kind="ExternalInput").ap()
    out_d = nc.dram_tensor("partial", [S, D], F32, kind="ExternalOutput").ap()

    Exp = mybir.ActivationFunctionType.Exp
    MULT = mybir.AluOpType.mult
    ADD = mybir.AluOpType.add
    SHUF_MASK = [i ^ 1 for i in range(32)]

    with tile.TileContext(nc) as tc:
        with (
            tc.tile_pool(name="constp", bufs=1) as constp,
            tc.tile_pool(name="wpool", bufs=1) as wpool,
            tc.tile_pool(name="xsp", bufs=46) as xsp,
            tc.tile_pool(name="qkvp", bufs=1) as qkvp,
            tc.tile_pool(name="workp", bufs=3) as workp,
            tc.tile_pool(name="epool", bufs=3) as epool,
            tc.tile_pool(name="empool", bufs=10) as empool,
            tc.tile_pool(name="drp", bufs=3) as drp,
            tc.tile_pool(name="outsp", bufs=8) as outsp,
            tc.tile_pool(name="psQS", bufs=3, space="PSUM") as psQS,
            tc.tile_pool(name="psPO", bufs=1, space="PSUM") as psPO,
            tc.tile_pool(name="psPP", bufs=2, space="PSUM") as psPP,
        ):
            # ---- constants ----
            cos_t = constp.tile([128, S], BF16)
            sin_t = constp.tile([128, S], BF16)
            mm_t = constp.tile([128, 256], BF16)
            md_t = constp.tile([128, 128], BF16)
            id64_t = constp.tile([64, 64], BF16)
            id128_t = constp.tile([128, 128], BF16)
            nc.sync.dma_start(cos_t[:], cos_d[:])
            nc.sync.dma_start(sin_t[:], sin_d[:])
            nc.sync.dma_start(mm_t[:], mm_d[:])
            nc.sync.dma_start(md_t[:], md_d[:])
            nc.sync.dma_start(id64_t[:], id64_d[:])
            nc.sync.dma_start(id128_t[:], id128_d[:])

            # ---- weight + first x chunk DMAs, issue-interleaved so the
            # first contraction tiles land first ----
            w_sb = []
            x_sb = [[None] * KT for _ in range(4)]
            for k in range(KT):
                w_t = wpool.tile([128, 640], BF16, name=f"w{k}")
                nc.sync.dma_start(w_t[:], w_d[128 * k : 128 * (k + 1), :])
                w_sb.append(w_t)
                x_t = xsp.tile([128, 512], BF16, name="x_t", tag="x")
                nc.sync.dma_start(x_t[:], xT_d[128 * k : 128 * (k + 1), 0:512])
                x_sb[0][k] = x_t
            wo_sb = []
            for t in range(4):
                w_t = wpool.tile([128, D], BF16, name=f"wo{t}")
                nc.sync.dma_start(w_t[:], wo_d[128 * t : 128 * (t + 1), :])
                wo_sb.append(w_t)

            def prefetch_x(sq):
                for k in range(KT):
                    x_t = xsp.tile([128, 512], BF16, name="x_t", tag="x")
                    nc.sync.dma_start(
                        x_t[:], xT_d[128 * k : 128 * (k + 1), 512 * sq : 512 * (sq + 1)]
                    )
                    x_sb[sq][k] = x_t

            # ---- persistent bf16 activation tensors ----
            qTm = [qkvp.tile([128, S], BF16, name=f"qTm{t}") for t in range(4)]
            kT2 = qkvp.tile([128, S], BF16, name="kT2")
            vT = qkvp.tile([64, S], BF16, name="vT")
            # v_lo = [v | 0], v_hi = [0 | v]: AV for an even head lands in
            # PSUM rows 0:64, its odd partner in rows 64:128 of the same
            # region -> head pairs stack in place, one copy per pair
            v_lo = [qkvp.tile([128, 128], BF16, name=f"vl{i}") for i in range(NQT)]
            v_hi = [qkvp.tile([128, 128], BF16, name=f"vh{i}") for i in range(NQT)]
            # single attention-output tile [128, 4 m-tiles, S]: po lands with
            # ONE strided copy per query tile; O-proj slices [:, t, seq]
            attn_oT = qkvp.tile([128, 4, S], BF16, name="attn_oT")
            for i in range(NQT):
                nc.vector.memset(v_lo[i][:, 64:128], 0.0)
                nc.vector.memset(v_hi[i][:, 0:64], 0.0)

            def rope_q(ps, mt, c0):
                # DVE casts the PSUM once; the SBUF-only bf16 muls/adds go
                # to the idle GpSimd engine
                t_all = workp.tile([128, 512], BF16, tag="ra", name="t_all")
                nc.vector.tensor_copy(t_all[:], ps[:])
                t_shuf = workp.tile([128, 512], BF16, tag="rb", name="t_shuf")
                nc.vector.stream_shuffle(t_shuf[:], t_all[:], SHUF_MASK)
                t_cos = workp.tile([128, 512], BF16, tag="rc", name="t_cos")
                nc.gpsimd.tensor_mul(t_cos[:], t_all[:], cos_t[:, c0 : c0 + 512])
                t_sin = workp.tile([128, 512], BF16, tag="rd", name="t_sin")
                nc.vector.tensor_mul(t_sin[:], t_shuf[:], sin_t[:, c0 : c0 + 512])
                nc.vector.tensor_add(qTm[mt][:, c0 : c0 + 512], t_cos[:], t_sin[:])

            def rope_kv(ps, c0):
                t_all = workp.tile([128, 512], BF16, tag="ra", name="t_allk")
                nc.vector.tensor_copy(t_all[0:64, :], ps[0:64, :])
                t_shuf = workp.tile([128, 512], BF16, tag="rb", name="t_shufk")
                nc.vector.stream_shuffle(t_shuf[0:64, :], t_all[0:64, :], SHUF_MASK)
                t_cos = workp.tile([128, 512], BF16, tag="rc", name="t_cosk")
                nc.gpsimd.tensor_mul(
                    t_cos[0:64, :], t_all[0:64, :], cos_t[0:64, c0 : c0 + 512]
                )
                t_sin = workp.tile([128, 512], BF16, tag="rd", name="t_sink")
                nc.vector.tensor_mul(
                    t_sin[0:64, :], t_shuf[0:64, :], sin_t[0:64, c0 : c0 + 512]
                )
                nc.vector.tensor_add(
                    kT2[0:64, c0 : c0 + 512], t_cos[0:64, :], t_sin[0:64, :]
                )
                # partition-shifted write stays on the vector engine
                nc.vector.tensor_add(
                    kT2[64:128, c0 : c0 + 512], t_cos[0:64, :], t_sin[0:64, :]
                )
                nc.vector.tensor_copy(vT[:, c0 : c0 + 512], ps[64:128, :])

            def emit_oproj(j, ch0, ch1):
                for ch in range(ch0, ch1):
                    pp = psPP.tile([128, OCH], F32, name="pp", tag="pp")
                    for t in range(4):
                        nc.tensor.matmul(
                            pp[:],
                            attn_oT[:, t, 128 * j : 128 * (j + 1)],
                            wo_sb[t][:, OCH * ch : OCH * (ch + 1)],
                            start=(t == 0),
                            stop=(t == 3),
                        )
                    osb = outsp.tile([128, OCH], F32, tag="osb", name="osb")
                    nc.scalar.copy(osb[:], pp[:])
                    nc.sync.dma_start(
                        out_d[128 * j : 128 * (j + 1), OCH * ch : OCH * (ch + 1)],
                        osb[:],
                    )

            for sq in range(4):
                c0 = 512 * sq
                if sq < 3:
                    prefetch_x(sq + 1)
                # ---- QKV projections, mt-major ----
                for mt in range(4):
                    ps = psQS.tile([128, 512], F32, tag="sp", name="ps_q")
                    for k in range(KT):
                        nc.tensor.matmul(
                            ps[:],
                            w_sb[k][:, 128 * mt : 128 * (mt + 1)],
                            x_sb[sq][k][:],
                            start=(k == 0),
                            stop=(k == KT - 1),
                        )
                    rope_q(ps, mt, c0)
                ps = psQS.tile([128, 512], F32, tag="sp", name="ps_kv")
                for k in range(KT):
                    nc.tensor.matmul(
                        ps[:],
                        w_sb[k][:, 512:640],
                        x_sb[sq][k][:],
                        start=(k == 0),
                        stop=(k == KT - 1),
                    )
                rope_kv(ps, c0)
                # ---- v transposes to natural [seq, hd] -> v_lo / v_hi ----
                for i in range(4 * sq, 4 * sq + 4):
                    tr = psQS.tile([128, 64], BF16, tag="sp", name="vtr")
                    nc.tensor.transpose(
                        tr[:], vT[:, 128 * i : 128 * (i + 1)], id64_t[:]
                    )
                    nc.scalar.copy(v_lo[i][:, 0:64], tr[:])
                    nc.scalar.copy(v_hi[i][:, 64:128], tr[:])

                # ---- attention for the 4 query tiles of this chunk ----
                for j in range(4 * sq, 4 * sq + 4):
                    W = 128 if j == 0 else 256
                    kc0 = 0 if j == 0 else 128 * (j - 1)
                    mask = md_t if j == 0 else mm_t
                    den = drp.tile([128, 8], F32, tag="den", name="den")
                    e_pairs = []
                    for h in range(8):
                        t, rb = h // 2, 64 * (h % 2)
                        hp = h % 2
                        if hp == 0:
                            s_pair = psQS.tile(
                                [128, 512], F32, tag="sp", name="s_pair"
                            )
                        sl = s_pair[:, 256 * hp : 256 * hp + W]
                        nc.tensor.matmul(
                            sl,
                            qTm[t][rb : rb + 64, 128 * j : 128 * (j + 1)],
                            kT2[rb : rb + 64, kc0 : kc0 + W],
                            start=True,
                            stop=False,
                        )
                        # additive -1e30 window mask via id128^T @ mask
                        nc.tensor.matmul(
                            sl,
                            id128_t[:],
                            mask[:, 0:W],
                            start=False,
                            stop=True,
                        )
                        if hp == 1:
                            # one wide masked exp per pair, straight from PSUM
                            e_pair = empool.tile(
                                [128, 512], BF16, tag="em", name="e_pair"
                            )
                            if W == 256:
                                nc.scalar.activation(
                                    e_pair[:], s_pair[:], Exp, scale=0.125
                                )
                            else:
                                nc.scalar.activation(
                                    e_pair[:, 0:128], s_pair[:, 0:128],
                                    Exp, scale=0.125,
                                )
                                nc.scalar.activation(
                                    e_pair[:, 256:384], s_pair[:, 256:384],
                                    Exp, scale=0.125,
                                )
                            e_pairs.append(e_pair)
                            # softmax denominators on the vector engine
                            nc.vector.reduce_sum(
                                out=den[:, h - 1 : h],
                                in_=e_pair[:, 0:W],
                                axis=mybir.AxisListType.X,
                            )
                            nc.vector.reduce_sum(
                                out=den[:, h : h + 1],
                                in_=e_pair[:, 256 : 256 + W],
                                axis=mybir.AxisListType.X,
                            )
                        # PE filler between score pairs: previous O-projection
                        if j > 0:
                            if h == 3:
                                emit_oproj(j - 1, 0, 3)
                            elif h == 7:
                                emit_oproj(j - 1, 3, 6)
                    rec = drp.tile([128, 8], F32, tag="rec", name="rec")
                    nc.vector.reciprocal(rec[:], den[:])
                    # one [128,512] PSUM region holds all 8 heads: pair
                    # (2t, 2t+1) stacks into rows 0:64 / 64:128 of quarter t
                    po = psPO.tile([128, 512], F32, tag="po", name="po")
                    for h in range(8):
                        t = h // 2
                        hp = h % 2  # position within the eT pair
                        e_n = epool.tile([128, 256], BF16, tag="en", name="e_n")
                        nc.vector.tensor_scalar_mul(
                            e_n[:, 0:W],
                            e_pairs[h // 2][:, 256 * hp : 256 * hp + W],
                            rec[:, h : h + 1],
                        )
                        if hp == 0:
                            eT_ps = psQS.tile([128, 512], BF16, tag="et", name="eT_ps", bufs=2)
                        for half in range(W // 128):
                            nc.tensor.transpose(
                                eT_ps[:, 256 * hp + 128 * half : 256 * hp + 128 * (half + 1)],
                                e_n[:, 128 * half : 128 * (half + 1)],
                                id128_t[:],
                            )
                        if hp == 1:
                            eT_sb = epool.tile(
                                [128, 512], BF16, tag="et", name="eT_sb"
                            )
                            if W == 256:
                                nc.vector.tensor_copy(eT_sb[:], eT_ps[:])
                            else:
                                nc.vector.tensor_copy(
                                    eT_sb[:, 0:128], eT_ps[:, 0:128]
                                )
                                nc.vector.tensor_copy(
                                    eT_sb[:, 256:384], eT_ps[:, 256:384]
                                )
                            # AV for the pair: even head -> rows 0:64 (v_lo),
                            # odd head -> rows 64:128 (v_hi), same quarter
                            for h2 in (h - 1, h):
                                hp2 = h2 % 2
                                vv = [v_lo, v_hi][hp2]
                                dst = po[:, 128 * t : 128 * (t + 1)]
                                if j > 0:
                                    nc.tensor.matmul(
                                        dst,
                                        vv[j - 1][:],
                                        eT_sb[:, 256 * hp2 : 256 * hp2 + 128],
                                        start=(hp2 == 0),
                                        stop=False,
                                    )
                                    nc.tensor.matmul(
                                        dst,
                                        vv[j][:],
                                        eT_sb[:, 256 * hp2 + 128 : 256 * hp2 + 256],
                                        start=False,
                                        stop=(hp2 == 1),
                                    )
                                else:
                                    nc.tensor.matmul(
                                        dst,
                                        vv[0][:],
                                        eT_sb[:, 256 * hp2 : 256 * hp2 + 128],
                                        start=(hp2 == 0),
                                        stop=(hp2 == 1),
                                    )
                    # one strided copy: po quarters -> the 4 m-tile planes
                    nc.scalar.copy(
                        attn_oT[:, :, 128 * j : 128 * (j + 1)],
                        po[:],
                    )
            emit_oproj(NQT - 1, 0, 6)
    nc.compile()
    return nc


def _prep_inputs(x, rope_cache, wq_w, wq_b, wk_w, wk_b, wv_w, wv_b, wo_w):
    """Build the shared + per-core input maps (all bf16 on device)."""
    import ml_dtypes

    bf = ml_dtypes.bfloat16
    xT = np.zeros((DP, S), dtype=np.float32)
    xT[0:D, :] = np.ascontiguousarray(x[0].T)
    xT[D, :] = 1.0  # bias row
    xT = xT.astype(bf)

    cos = np.asarray(rope_cache[:, 0, :], dtype=np.float32)  # [S, 64]
    sin = np.asarray(rope_cache[:, 1, :], dtype=np.float32)
    cosP = cos[:, PERM].T  # [64, S] permuted head-dim rows
    sinP = sin[:, PERM].T
    sign = np.where(PERM < 32, -1.0, 1.0).astype(np.float32)[:, None]
    sinPs = sinP * sign
    cosT = np.concatenate([cosP, cosP], axis=0).astype(bf)  # [128, S]
    sinTs = np.concatenate([sinPs, sinPs], axis=0).astype(bf)

    # S-layout additive masks: query a on partitions, key b on free.
    a_idx = np.arange(128)[:, None]
    b_idx = np.arange(256)[None, :]
    left = (b_idx < 128) & (b_idx > a_idx)
    right = (b_idx >= 128) & ((b_idx - 128) <= a_idx)
    m_main = np.where(left | right, 0.0, -1e30).astype(bf)  # [128, 256], j >= 1
    m_diag = np.where(b_idx[:, :128] <= a_idx, 0.0, -1e30).astype(bf)  # j == 0

    id64 = np.eye(64, dtype=np.float32).astype(bf)
    id128 = np.eye(128, dtype=np.float32).astype(bf)

    shared = dict(
        xT=xT, cosT=cosT, sinTs=sinTs, m_main=m_main, m_diag=m_diag,
        id64=id64, id128=id128,
    )

    in_maps = []
    for c in range(N_CORES):
        # wq slice: q heads [8c, 8c+8), head-dim permuted, transposed, bias row
        wq_rows = []
        bq_rows = []
        for hh in range(8):
            g = 8 * c + hh
            wq_rows.append(wq_w[64 * g + PERM, :])  # [64, D]
            bq_rows.append(wq_b[64 * g + PERM])
        wq_slice = np.concatenate(wq_rows, axis=0)  # [512, D]
        bq_slice = np.concatenate(bq_rows, axis=0)  # [512]

        wk_slice = wk_w[64 * c + PERM, :]  # [64, D] permuted
        bk_slice = wk_b[64 * c + PERM]
        wv_slice = wv_w[64 * c : 64 * (c + 1), :]  # unpermuted
        bv_slice = wv_b[64 * c : 64 * (c + 1)]

        w_t = np.zeros((DP, 640), dtype=np.float32)
        w_t[0:D, 0:512] = wq_slice.T
        w_t[D, 0:512] = bq_slice
        w_t[0:D, 512:576] = wk_slice.T
        w_t[D, 512:576] = bk_slice
        w_t[0:D, 576:640] = wv_slice.T
        w_t[D, 576:640] = bv_slice

        wo_t = np.ascontiguousarray(
            wo_w[:, 512 * c : 512 * (c + 1)].T
        ).astype(bf)  # [512, D]

        in_maps.append(dict(shared, wqkv=w_t.astype(bf), wo=wo_t))
    return in_maps


def kernel(
    x,
    rope_cache,
    wq_w,
    wq_b,
    wk_w,
    wk_b,
    wv_w,
    wv_b,
    wo_w,
    wo_b,
):
    global _COMPILED
    x = np.asarray(x, dtype=np.float32)
    rope_cache = np.asarray(rope_cache, dtype=np.float32)
    wq_w = np.asarray(wq_w, dtype=np.float32)
    wq_b = np.asarray(wq_b, dtype=np.float32)
    wk_w = np.asarray(wk_w, dtype=np.float32)
    wk_b = np.asarray(wk_b, dtype=np.float32)
    wv_w = np.asarray(wv_w, dtype=np.float32)
    wv_b = np.asarray(wv_b, dtype=np.float32)
    wo_w = np.asarray(wo_w, dtype=np.float32)
    wo_b = np.asarray(wo_b, dtype=np.float32)

    if _COMPILED is None:
        _COMPILED = _build()
    nc = _COMPILED

    in_maps = _prep_inputs(x, rope_cache, wq_w, wq_b, wk_w, wk_b, wv_w, wv_b, wo_w)
    res = run_bass_kernel_spmd(nc, in_maps, core_ids=list(range(N_CORES)), trace=False)
    out = np.zeros((S, D), dtype=np.float32)
    for c in range(N_CORES):
        out += res.results[c]["partial"]
    out += wo_b[None, :]
    return out.reshape(B, S, D).astype(np.float32)


# expose the compiled-module runner for test harnesses that want tracing
def run_traced(**inputs):
    global _COMPILED
    if _COMPILED is None:
        _COMPILED = _build()
    in_maps = _prep_inputs(
        np.asarray(inputs["x"], np.float32),
        np.asarray(inputs["rope_cache"], np.float32),
        np.asarray(inputs["wq_w"], np.float32),
        np.asarray(inputs["wq_b"], np.float32),
        np.asarray(inputs["wk_w"], np.float32),
        np.asarray(inputs["wk_b"], np.float32),
        np.asarray(inputs["wv_w"], np.float32),
        np.asarray(inputs["wv_b"], np.float32),
        np.asarray(inputs["wo_w"], np.float32),
    )
    res = run_bass_kernel_spmd(
        _COMPILED, in_maps, core_ids=list(range(N_CORES)), trace=True
    )
    out = np.zeros((S, D), dtype=np.float32)
    for c in range(N_CORES):
        out += res.results[c]["partial"]
    out += np.asarray(inputs["wo_b"], np.float32)[None, :]
    return out.reshape(B, S, D).astype(np.float32), res
